# revision 1
# baseline (speedup 1.0000x reference)
"""Sparse (tanh-clipped, key-masked) dot-product attention on 8 trn2 NeuronCores.

Reference computation (per batch b, head h):
    logits = (Q @ K^T) / 8
    logits = 10 * tanh(logits)
    logits[masked keys] = -inf          (mask is per (batch, key))
    out = softmax(logits) @ V

Strategy:
  - Host: gather K/V down to the unmasked keys (~50% of 2048), pad to a
    common multiple of 128 across batches.  Pre-transpose Q and K to
    [64, S] so the contraction dim (d=64) is on partitions.  Pad keys get
    an exp() bias of -100 so they contribute exp(10*tanh + -100) ~= 0.
  - Device (per core: one batch, 8 heads), two pipelined phases per head:
    phase 1 (per 128-key tile t):
      PE:  S_T[k, q] = Kt-tile^T @ Qt      (float32r, 1 cycle/row)
      ACT: tanh(s/8) -> exp(10*t + bias)   -> P[t] in bf16
    phase 2 (per 128-query block, overlapped with the NEXT head's phase 1):
      PE:  out_q[128q, 65] = sum_t P[t][:, q-block]^T @ [V|1]-tile (bf16)
      DVE: reciprocal of the ones-column, scale, DMA out — output lands in
      natural [q, d] layout with no transposes at all.
  - softmax needs no max-subtraction: 10*tanh is bounded in [-10, 10].
  - probs are bf16: numerator and denominator use the same rounded p, so
    bf16 rounding cancels to first order in the normalized output.
"""

import sys

if "/opt/trn_rl_repo" not in sys.path:
    sys.path.insert(0, "/opt/trn_rl_repo")

import ml_dtypes
import numpy as np

import concourse.tile as tile
from concourse import bacc, mybir
from concourse.bass_utils import run_bass_kernel_spmd

B, H, S, D = 4, 16, 2048, 64
N_CORES = 8
HPC = B * H // N_CORES  # heads per core = 8 (each core: 1 batch, 8 heads)
Q_CHUNK = 512  # PSUM-bank / fp32 moving-operand limit
F32 = mybir.dt.float32
F32R = mybir.dt.float32r
BF16 = mybir.dt.bfloat16

_kernel_cache = {}


def _build_kernel(n_kp: int, reps: int = 1):
    """Build the per-core Bass program for n_kp (padded) kept keys.

    reps > 1 repeats the whole computation (for overhead-free timing).
    """
    n_kt = n_kp // 128
    nc = bacc.Bacc(None)

    qt_p = nc.declare_dram_parameter("qt", [HPC, D, S], F32R, isOutput=False)
    kt_p = nc.declare_dram_parameter("kt", [HPC, D, n_kp], F32R, isOutput=False)
    v_p = nc.declare_dram_parameter("vaug", [HPC, 128, n_kt, D + 1], BF16, isOutput=False)
    bias_p = nc.declare_dram_parameter("bias", [128, n_kt], F32, isOutput=False)
    out_p = nc.declare_dram_parameter("out", [HPC, S, D], F32, isOutput=True)

    n_qc = S // Q_CHUNK  # 4 q-chunks of 512
    n_qi = S // 128  # 16 query row-blocks

    with tile.TileContext(nc) as tc:
        with (
            tc.tile_pool(name="consts", bufs=1) as consts,
            tc.tile_pool(name="inq", bufs=2) as inq,
            tc.tile_pool(name="ink", bufs=2) as ink,
            tc.tile_pool(name="inv", bufs=2) as inv,
            tc.tile_pool(name="act", bufs=2) as act_pool,
            # two headfuls of P tiles for full phase-1/phase-2 overlap;
            # capped for unusually low mask density (SBUF budget)
            tc.tile_pool(
                name="probs", bufs=2 * n_kt + 1 if n_kt <= 10 else n_kt + 4
            ) as probs_pool,
            tc.tile_pool(name="outsb", bufs=6) as out_pool,
            tc.tile_pool(name="ps_st", bufs=1, space="PSUM") as ps_st,
            tc.tile_pool(name="ps_oq", bufs=4, space="PSUM") as ps_oq,
        ):
            bias_sb = consts.tile([128, n_kt], F32)
            nc.sync.dma_start(out=bias_sb, in_=bias_p[:])
            # prime the ACT exp_and_others table set (tanh+exp) while the
            # first input DMAs are still in flight
            warm = consts.tile([128, 1], F32)
            nc.vector.memset(warm, 0.0)
            nc.scalar.activation(warm, warm, mybir.ActivationFunctionType.Tanh)
            nc.scalar.activation(warm, warm, mybir.ActivationFunctionType.Exp)

            def phase2_group(hh, qi, p_tiles, v_tile):
                oq_ps = ps_oq.tile([128, D + 1], F32, tag="oq")
                for j in range(n_kt):
                    nc.tensor.matmul(
                        oq_ps,
                        lhsT=p_tiles[j][:, qi * 128 : (qi + 1) * 128],
                        rhs=v_tile[:, j, :],
                        start=(j == 0),
                        stop=(j == n_kt - 1),
                    )
                recip = out_pool.tile([128, 1], F32, tag="recip")
                nc.vector.reciprocal(recip, oq_ps[:, D : D + 1])
                oq = out_pool.tile([128, D], F32, tag="out")
                nc.vector.tensor_scalar_mul(oq, oq_ps[:, 0:D], recip)
                nc.sync.dma_start(
                    out=out_p[hh, qi * 128 : (qi + 1) * 128, :], in_=oq
                )

            heads = [h for _ in range(reps) for h in range(HPC)]
            prev = None  # (head, p_tiles, v_tile) pending phase 2
            # spread the previous head's 16 phase-2 groups over this head's
            # key-tile steps t=1..n_kt-1 (any leftovers drain after the loop)
            per_step = -(-n_qi // max(1, n_kt - 1))
            for i, h in enumerate(heads):
                qt_sb = inq.tile([D, S], F32R, tag="qt")
                kt_sb = ink.tile([D, n_kp], F32R, tag="kt")
                if i == 0:
                    # fine-grained first loads so the first matmul (and the
                    # ACT pipe behind it) starts as early as possible
                    nc.sync.dma_start(
                        out=kt_sb[:, 0:128], in_=kt_p[h][:, 0:128]
                    )
                    for qc in range(n_qc):
                        nc.sync.dma_start(
                            out=qt_sb[:, qc * Q_CHUNK : (qc + 1) * Q_CHUNK],
                            in_=qt_p[h][:, qc * Q_CHUNK : (qc + 1) * Q_CHUNK],
                        )
                    if n_kp > 128:
                        nc.sync.dma_start(
                            out=kt_sb[:, 128:], in_=kt_p[h][:, 128:]
                        )
                else:
                    nc.sync.dma_start(out=qt_sb, in_=qt_p[h])
                    nc.sync.dma_start(out=kt_sb, in_=kt_p[h])
                v_sb = inv.tile([128, n_kt, D + 1], BF16, tag="v")
                nc.sync.dma_start(out=v_sb, in_=v_p[h])

                p_tiles = []
                qi_cursor = 0
                for t in range(n_kt):
                    st_ps = ps_st.tile([128, S], F32, tag="st")
                    for qc in range(n_qc):
                        nc.tensor.matmul(
                            st_ps[:, qc * Q_CHUNK : (qc + 1) * Q_CHUNK],
                            lhsT=kt_sb[:, t * 128 : (t + 1) * 128],
                            rhs=qt_sb[:, qc * Q_CHUNK : (qc + 1) * Q_CHUNK],
                            start=True,
                            stop=True,
                        )
                    t_sb = act_pool.tile([128, S], F32, tag="tanh")
                    nc.scalar.activation(
                        t_sb,
                        st_ps,
                        mybir.ActivationFunctionType.Tanh,
                        scale=0.125,
                    )
                    p_sb = probs_pool.tile([128, S], BF16, tag="p")
                    nc.scalar.activation(
                        p_sb,
                        t_sb,
                        mybir.ActivationFunctionType.Exp,
                        bias=bias_sb[:, t : t + 1],
                        scale=10.0,
                    )
                    p_tiles.append(p_sb)
                    # overlap: drain the previous head's phase 2 under this
                    # head's ACT work
                    if prev is not None and t >= 1:
                        for _ in range(min(per_step, n_qi - qi_cursor)):
                            phase2_group(prev[0], qi_cursor, prev[1], prev[2])
                            qi_cursor += 1
                if prev is not None:
                    for qi in range(qi_cursor, n_qi):
                        phase2_group(prev[0], qi, prev[1], prev[2])
                prev = (h, p_tiles, v_sb)
            for qi in range(n_qi):
                phase2_group(prev[0], qi, prev[1], prev[2])
    if not nc.is_finalized():
        nc.finalize()
    return nc


def _prep_inputs(q, k, v, mask):
    """Host-side shard + gather + layout. Returns (in_maps, n_kp)."""
    keep = [np.flatnonzero(~mask[b, :, 0]) for b in range(B)]
    n_kp = max(128, -(-max(len(kb) for kb in keep) // 128) * 128)
    n_kt = n_kp // 128

    in_maps = []
    for c in range(N_CORES):
        b = c // 2
        h0 = (c % 2) * HPC
        kb = keep[b]
        nk = len(kb)

        qt = np.ascontiguousarray(q[b, h0 : h0 + HPC].transpose(0, 2, 1))

        kg = np.zeros((HPC, n_kp, D), np.float32)
        kg[:, :nk] = k[b, h0 : h0 + HPC][:, kb]
        kt = np.ascontiguousarray(kg.transpose(0, 2, 1))

        vg = np.zeros((HPC, n_kp, D + 1), np.float32)
        vg[:, :nk, :D] = v[b, h0 : h0 + HPC][:, kb]
        vg[:, :, D] = 1.0
        # [HPC, n_kt, 128, 65] -> [HPC, 128, n_kt, 65] (partition-major)
        vaug = np.ascontiguousarray(
            vg.reshape(HPC, n_kt, 128, D + 1).transpose(0, 2, 1, 3)
        ).astype(ml_dtypes.bfloat16)

        bias = np.zeros((128, n_kt), np.float32)
        idx = np.arange(n_kp).reshape(n_kt, 128).T  # [128, n_kt]
        bias[idx >= nk] = -100.0

        in_maps.append({"qt": qt, "kt": kt, "vaug": vaug, "bias": bias})
    return in_maps, n_kp


def kernel(q, k, v, mask, _trace=False):
    q = np.asarray(q, np.float32)
    k = np.asarray(k, np.float32)
    v = np.asarray(v, np.float32)
    mask = np.asarray(mask, bool)
    assert q.shape == k.shape == v.shape == (B, H, S, D), (q.shape,)
    assert mask.shape == (B, S, 1), (mask.shape,)

    in_maps, n_kp = _prep_inputs(q, k, v, mask)
    if n_kp not in _kernel_cache:
        _kernel_cache[n_kp] = _build_kernel(n_kp)
    nc = _kernel_cache[n_kp]

    # a core occasionally comes up wedged (NRT_EXEC_UNIT_UNRECOVERABLE,
    # self-recovers in ~30 s) — retry rather than fail the whole call
    import time as _time

    res = None
    for attempt in range(3):
        try:
            res = run_bass_kernel_spmd(
                nc, in_maps, list(range(N_CORES)), trace=_trace
            )
            break
        except Exception:
            if attempt == 2:
                raise
            _time.sleep(30)
    out = np.empty((B, H, S, D), np.float32)
    for c in range(N_CORES):
        b = c // 2
        h0 = (c % 2) * HPC
        out[b, h0 : h0 + HPC] = res.results[c]["out"]
    if _trace:
        return out, res
    return out


if __name__ == "__main__":
    rng = np.random.default_rng(0)
    q = rng.standard_normal((B, H, S, D), np.float32)
    k = rng.standard_normal((B, H, S, D), np.float32)
    v = rng.standard_normal((B, H, S, D), np.float32)
    mask = rng.integers(0, 2, (B, S, 1)).astype(bool)
    out = kernel(q, k, v, mask)
    print("out", out.shape, out.dtype, float(np.abs(out).max()))

